# revision 1
# baseline (speedup 1.0000x reference)
"""Trainium2 Bass kernel for nn_ContrastiveLoss (NT-Xent with sampled negatives).

Reference semantics (B=4096, D=512, N=8192, R=4 negatives/row, temp=0.5+1e-8):
    z  = concat(z_i, z_j)                       [N, D]
    zn = z / max(||z||, 1e-8)
    sim = (zn @ zn.T) / temp
    pos[i]  = sim[i, (i+B) % N]
    cols    = neg_idx + (neg_idx >= row)        (skip-diagonal remap)
    neg[i,k] = sim[i, cols[i,k]]
    nll = logsumexp([pos, neg]) - pos ;  loss = mean(nll)

Key insight: only 5 entries of each sim row are needed, so we never form the
[N, N] matrix. Each of the 8 cores takes a 1024-row slab, gathers the 5
partner rows per row (1 static positive slab + 4 indirect-DMA gathers),
computes cosine dots with fused DVE tensor_tensor_reduce ops, norms with
fused ACT square+accum, then a 5-wide log-softmax and a partial sum.
Host sums the 8 partials.
"""

import os
import sys

import numpy as np

if "/opt/trn_rl_repo" not in sys.path:
    sys.path.insert(0, "/opt/trn_rl_repo")

B = 4096
D = 512
N = 2 * B
R = 4  # negatives per row
NCORES = 8
RPC = N // NCORES  # rows per core = 1024
P = 128  # partitions
J = RPC // P  # row-tiles per core = 8
TEMP = 0.5 + 1e-08
EPS = 1e-08
INV_TEMP = float(1.0 / TEMP)

_CACHE = {}


def build_nc():
    import concourse.bass as bass
    import concourse.bacc as bacc
    import concourse.mybir as mybir
    from concourse.tile import TileContext

    fp32 = mybir.dt.float32
    i32 = mybir.dt.int32

    # Bacc (not raw Bass): its compile pipeline legalizes TRN2's
    # one-sync-wait-per-instruction constraint via event semaphores.
    nc = bacc.Bacc()
    z_full = nc.dram_tensor("z_full", [N, D], fp32, kind="ExternalInput")
    # own rows followed by positive-partner rows, one DMA -> one wait
    zop = nc.dram_tensor("zop", [2 * RPC, D], fp32, kind="ExternalInput")
    # neg indices [P, R, J] followed by row ids [P, 1, J]
    idx = nc.dram_tensor("idx", [P, R + 1, J], i32, kind="ExternalInput")
    out_partial = nc.dram_tensor("partial", [1, 1], fp32, kind="ExternalOutput")
    dbg = os.environ.get("K_DEBUG", "0") == "1"
    if dbg:
        out_logit = nc.dram_tensor(
            "logit_out", [P, J, 1 + R], fp32, kind="ExternalOutput"
        )
        out_cols = nc.dram_tensor("cols_out", [P, R, J], i32, kind="ExternalOutput")
        out_g = nc.dram_tensor("g_out", [P, R * J * D], fp32, kind="ExternalOutput")

    AF = mybir.ActivationFunctionType
    OP = mybir.AluOpType

    with TileContext(nc) as tc:
        with (
            tc.tile_pool(name="big", bufs=1) as big,
            tc.tile_pool(name="small", bufs=1) as small,
            tc.tile_pool(name="scr", bufs=6) as scr,
            tc.tile_pool(name="psum", bufs=4, space="PSUM") as pp,
        ):
            # ---- bulk load (own rows + positive-partner rows, single DMA) ----
            ap_t = big.tile([P, 2 * J, D], fp32, tag="AP")
            # row r_local = t*128 + p  ->  tile[p, t, :]
            nc.sync.dma_start(
                out=ap_t[:], in_=zop[:].rearrange("(t p) d -> p t d", p=P)
            )
            a_t = ap_t[:, 0:J, :]
            p_t = ap_t[:, J : 2 * J, :]

            # ---- index prep: cols = neg + (neg >= row), laid out [P, R, J] so
            # each k-slice is contiguous for the indirect-DMA offset AP ----
            nr = small.tile([P, R + 1, J], i32, tag="nr")
            nc.sync.dma_start(out=nr[:], in_=idx[:])
            ni = nr[:, 0:R, :]
            ri = nr[:, R : R + 1, :]
            ge = small.tile([P, R, J], i32, tag="ge")
            cols = small.tile([P, R, J], i32, tag="cols")
            nc.vector.tensor_tensor(
                out=ge[:], in0=ni, in1=ri.to_broadcast([P, R, J]), op=OP.is_ge
            )
            nc.vector.tensor_tensor(out=cols[:], in0=ni, in1=ge[:], op=OP.add)

            # ---- negative-row gathers: HW indirect DMA honors ONE index per
            # dest partition row (multi-index-per-partition is sim-only), so
            # issue one [P,1]-index gather per (k, j): 32 DMAs of 128 rows ----
            g_all = big.tile([P, R, J, D], fp32, tag="G")
            for k in range(R):
                for j in range(J):
                    nc.gpsimd.indirect_dma_start(
                        out=g_all[:, k, j, :],
                        out_offset=None,
                        in_=z_full[:],
                        in_offset=bass.IndirectOffsetOnAxis(
                            ap=cols[:, k, j : j + 1], axis=0
                        ),
                    )
            g_t = [g_all[:, k, :, :] for k in range(R)]

            # ---- row sum-of-squares (ACT square with fused row-sum) ----
            ssa = small.tile([P, J, 1], fp32, tag="ssa")
            ssp = small.tile([P, J, 1], fp32, tag="ssp")
            ssg = small.tile([P, J, R], fp32, tag="ssg")
            for j in range(J):
                sq = scr.tile([P, D], fp32, tag="sq")
                nc.scalar.activation(
                    out=sq[:], in_=a_t[:, j, :], func=AF.Square,
                    accum_out=ssa[:, j, :],
                )
                sq = scr.tile([P, D], fp32, tag="sq")
                nc.scalar.activation(
                    out=sq[:], in_=p_t[:, j, :], func=AF.Square,
                    accum_out=ssp[:, j, :],
                )
                for k in range(R):
                    sq = scr.tile([P, D], fp32, tag="sq")
                    nc.scalar.activation(
                        out=sq[:], in_=g_t[k][:, j, :], func=AF.Square,
                        accum_out=ssg[:, j, k : k + 1],
                    )

            # ---- dots: one wide DVE multiply + one wide reduce per partner ----
            # (tensor_tensor_reduce is rejected by this walrus build, so
            # separate mult+reduce; wide [P, J*D] ops amortize issue overhead)
            from concourse.tile_rust import add_dep_helper

            # The TT ISA encoding has a single sync-wait slot, so each DVE
            # multiply may carry at most one semaphore wait: pin DVE order
            # (add_dep_helper) and give every partner its own product slot,
            # split in J-halves so slot reuse pairs only with an
            # already-observed DMA sem.
            J2 = J // 2
            dp = small.tile([P, J, 1], fp32, tag="dp")
            dg = small.tile([P, J, R], fp32, tag="dg")
            prev = None
            pairs = [(p_t, dp[:, :, 0:1], "pp")] + [
                (g_t[k], dg[:, :, k : k + 1], f"g{k}") for k in range(R)
            ]
            for x_ap, d_out, tag in pairs:
                for h in range(2):
                    js = slice(h * J2, (h + 1) * J2)
                    prod = big.tile([P, J2, D], fp32, tag=f"prod_{tag}")
                    mm = nc.vector.tensor_tensor(
                        out=prod[:], in0=a_t[:, js, :], in1=x_ap[:, js, :],
                        op=OP.mult,
                    )
                    if prev is not None:
                        add_dep_helper(mm.ins, prev.ins, sync=False,
                                       reason="dve-order")
                    prev = mm
                    nc.vector.tensor_reduce(
                        out=d_out[:, js, :], in_=prod[:],
                        axis=mybir.AxisListType.X, op=OP.add,
                    )

            # ---- inverse norms: inv = 1/max(sqrt(ss), eps) ----
            def inv_norm(ss, shape, tag, fold_temp):
                nrm = small.tile(shape, mybir.dt.float32, tag=tag + "_n")
                nc.scalar.sqrt(out=nrm[:], in_=ss[:])
                nc.vector.tensor_scalar(
                    out=nrm[:], in0=nrm[:], scalar1=float(EPS), scalar2=None,
                    op0=OP.max,
                )
                inv = small.tile(shape, mybir.dt.float32, tag=tag + "_i")
                nc.vector.reciprocal(out=inv[:], in_=nrm[:])
                if fold_temp:
                    nc.vector.tensor_scalar(
                        out=inv[:], in0=inv[:], scalar1=INV_TEMP, scalar2=None,
                        op0=OP.mult,
                    )
                return inv

            inva = inv_norm(ssa, [P, J, 1], "ia", fold_temp=True)  # has 1/temp
            invp = inv_norm(ssp, [P, J, 1], "ip", fold_temp=False)
            invg = inv_norm(ssg, [P, J, R], "ig", fold_temp=False)

            # ---- logits ----
            logit = small.tile([P, J, 1 + R], fp32, tag="logit")
            lp = logit[:, :, 0:1]
            lg = logit[:, :, 1 : 1 + R]
            nc.vector.tensor_tensor(out=lp, in0=dp[:], in1=inva[:], op=OP.mult)
            nc.vector.tensor_tensor(out=lp, in0=lp, in1=invp[:], op=OP.mult)
            nc.vector.tensor_tensor(
                out=lg, in0=dg[:], in1=inva[:].to_broadcast([P, J, R]), op=OP.mult
            )
            nc.vector.tensor_tensor(out=lg, in0=lg, in1=invg[:], op=OP.mult)

            # ---- 5-wide log-softmax:  nll = ln(sum(exp(l - m))) + m - lp ----
            mx = small.tile([P, J, 1], fp32, tag="mx")
            nc.vector.tensor_reduce(
                out=mx[:], in_=logit[:], axis=mybir.AxisListType.X, op=OP.max
            )
            lshift = small.tile([P, J, 1 + R], fp32, tag="lshift")
            nc.vector.tensor_tensor(
                out=lshift[:], in0=logit[:], in1=mx[:].to_broadcast([P, J, 1 + R]),
                op=OP.subtract,
            )
            ex = small.tile([P, J, 1 + R], fp32, tag="ex")
            nc.scalar.activation(out=ex[:], in_=lshift[:], func=AF.Exp)
            sume = small.tile([P, J, 1], fp32, tag="sume")
            nc.vector.tensor_reduce(
                out=sume[:], in_=ex[:], axis=mybir.AxisListType.X, op=OP.add
            )
            lns = small.tile([P, J, 1], fp32, tag="lns")
            nc.scalar.activation(out=lns[:], in_=sume[:], func=AF.Ln)
            nll = small.tile([P, J, 1], fp32, tag="nll")
            nc.vector.tensor_tensor(out=nll[:], in0=lns[:], in1=mx[:], op=OP.add)
            nc.vector.tensor_tensor(out=nll[:], in0=nll[:], in1=lp, op=OP.subtract)

            # ---- partial = sum over all 1024 rows (free-dim then partitions) ----
            rsum = small.tile([P, 1], fp32, tag="rsum")
            nc.vector.tensor_reduce(
                out=rsum[:], in_=nll[:], axis=mybir.AxisListType.XY, op=OP.add
            )
            ones = small.tile([P, 1], fp32, tag="ones")
            nc.vector.memset(ones[:], 1.0)
            psc = pp.tile([1, 1], fp32, tag="psc")
            nc.tensor.matmul(out=psc[:], lhsT=ones[:], rhs=rsum[:], start=True, stop=True)
            res = small.tile([1, 1], fp32, tag="res")
            nc.vector.tensor_copy(out=res[:], in_=psc[:])
            nc.sync.dma_start(out=out_partial[:], in_=res[:])
            if dbg:
                nc.sync.dma_start(out=out_logit[:], in_=logit[:])
                nc.sync.dma_start(out=out_cols[:], in_=cols[:])
                nc.sync.dma_start(
                    out=out_g[:], in_=g_all[:].rearrange("p r j d -> p (r j d)")
                )

    nc.finalize()  # runs Bacc.compile(): wait legalization + reg alloc
    return nc


def make_in_maps(z_i, z_j, neg_idx):
    z = np.ascontiguousarray(np.concatenate([z_i, z_j], axis=0), dtype=np.float32)
    neg_idx = np.asarray(neg_idx, dtype=np.int32)
    in_maps = []
    for m in range(NCORES):
        lo = m * RPC
        plo = (lo + B) % N
        # [RPC, R] -> [J, P, R] -> [P, R, J]
        ni = neg_idx[lo : lo + RPC].reshape(J, P, R).transpose(1, 2, 0)
        rows = np.arange(lo, lo + RPC, dtype=np.int32).reshape(J, P).T  # [P, J]
        idx = np.ascontiguousarray(
            np.concatenate([ni, rows[:, None, :]], axis=1)
        )
        zop = np.ascontiguousarray(
            np.concatenate([z[lo : lo + RPC], z[plo : plo + RPC]], axis=0)
        )
        in_maps.append({"z_full": z, "zop": zop, "idx": idx})
    return in_maps


def kernel(z_i, z_j, neg_idx, _bench=None):
    from concourse.bass_utils import run_bass_kernel_spmd

    if "nc" not in _CACHE:
        _CACHE["nc"] = build_nc()
    nc = _CACHE["nc"]
    in_maps = make_in_maps(z_i, z_j, neg_idx)
    core_ids = list(range(NCORES))
    kw = dict(_bench or {})
    r = run_bass_kernel_spmd(nc, in_maps, core_ids, **kw)
    if _bench is not None:
        _CACHE["last_results"] = r
    total = np.sum(
        [r.results[m]["partial"][0, 0] for m in range(NCORES)], dtype=np.float64
    )
    return np.float32(total / N)



# revision 2
# speedup vs baseline: 1.0218x; 1.0218x over previous
"""Trainium2 Bass kernel v3 for nn_ContrastiveLoss (NT-Xent, sampled negatives).

Reference (B=4096, D=512, N=8192, R=4, temp=0.5+1e-8):
    z  = concat(z_i, z_j);  zn = z / max(||z||, 1e-8)
    sim = (zn @ zn.T) / temp
    pos[i] = sim[i, (i+B)%N];  cols = neg_idx + (neg_idx >= row)
    neg[i,k] = sim[i, cols[i,k]]
    nll = logsumexp([pos, neg]) - pos;  loss = mean(nll)

Design:
  * Paired decomposition: core m owns z_i rows [m*512,(m+1)*512) AND their
    positive partners in z_j, so positives are in-slab (no partner gathers).
  * Everything bf16 + TRANSPOSED layout [d%128 partitions, d//128, row]:
    negatives arrive via Pool dma_gather(transpose=True) per (row-half, k)
    chunk; own slab is host-pre-transposed and DMA'd on SP.
  * Row-dot reductions (dots + squared-norms) become PE ones-matmuls:
    psum[:,col] accumulates lhsT=chunk [128d,128rows] x rhs=ones[128,1]
    over the 4 d-chunks -- near-free on the tensor engine, landing results
    row-major [row-partition, col] with no diagonal extraction.
  * Products (a*g) on DVE (bf16 2x mode), squares (g*g) split DVE/ACT.
  * One explicit ACT table load (natural_log_exp_and_others covers
    Square/Exp/Ln); inv-norms via exp(-0.5*ln(x)) so no Sqrt table needed.
  * Per-half PSUM tiles + per-half softmax so the first half's tail chain
    overlaps the second half's gathers; last gather split small to shrink
    the serial tail.
  * Logits bounded by 1/temp (cosine sim): softmax skips max-shift;
    exp(pos-logit) precomputed early, negatives exp'd per half.
  * cols remap (neg + (neg>=row)) stays on-device (int16 DVE ops).
"""

import sys

import numpy as np

if "/opt/trn_rl_repo" not in sys.path:
    sys.path.insert(0, "/opt/trn_rl_repo")

B = 4096
D = 512
N = 2 * B
R = 4
NCORES = 8
OWN = 1024            # rows per core (512 z_i + 512 paired z_j)
HLF = OWN // 2        # 512
P = 128
C = D // P            # 4 d-chunks
JT = OWN // P         # 8 j-tiles per core
TEMP = 0.5 + 1e-08
LN_TEMP = float(np.log(TEMP))

_CACHE = {}


def build_nc():
    import concourse.bacc as bacc
    import concourse.mybir as mybir
    from concourse.tile import TileContext

    fp32 = mybir.dt.float32
    bf16 = mybir.dt.bfloat16
    i16 = mybir.dt.int16

    AF = mybir.ActivationFunctionType
    OP = mybir.AluOpType

    nc = bacc.Bacc()
    zbf = nc.dram_tensor("zbf", [N, D], bf16, kind="ExternalInput")
    atp_d = nc.dram_tensor("atp", [P, C, OWN], bf16, kind="ExternalInput")
    # idx channels: 0..7 = neg idx for (h,k) chunk h*4+k; 8..9 = row ids (h)
    idx_d = nc.dram_tensor("idx", [P, 10, HLF // 16], i16, kind="ExternalInput")
    out_nll = nc.dram_tensor("nll", [P, 16], fp32, kind="ExternalOutput")

    S = HLF // 16  # 32 idx positions per partition lane

    with TileContext(nc) as tc:
        with (
            tc.tile_pool(name="big", bufs=1) as big,
            tc.tile_pool(name="small", bufs=1) as small,
            tc.tile_pool(name="pp", bufs=1, space="PSUM") as pp,
        ):
            # ---------------- load phase (SP queue) ----------------
            idxt = small.tile([P, 10, S], i16, tag="idxt")
            nc.sync.dma_start(out=idxt[:], in_=idx_d[:])
            atp = big.tile([P, C, OWN], bf16, tag="atp")
            nc.sync.dma_start(out=atp[:, :, 0:HLF], in_=atp_d[:, :, 0:HLF])
            nc.sync.dma_start(out=atp[:, :, HLF:OWN], in_=atp_d[:, :, HLF:OWN])

            # ---------------- index remap (DVE, int16) ----------------
            ni = idxt[:, 0:8, :]                      # [P, (h,k), S]
            ge = small.tile([P, 8, S], i16, tag="ge")
            cols = small.tile([P, 8, S], i16, tag="cols")
            # chunk (0,0) remapped first so its gather launches ASAP
            r0 = idxt[:, 8:9, :]
            r1 = idxt[:, 9:10, :]
            nc.vector.tensor_tensor(
                out=ge[:, 0:1, :], in0=ni[:, 0:1, :], in1=r0, op=OP.is_ge
            )
            nc.vector.tensor_tensor(
                out=cols[:, 0:1, :], in0=ni[:, 0:1, :], in1=ge[:, 0:1, :],
                op=OP.add,
            )
            nc.vector.tensor_tensor(
                out=ge[:, 1:4, :], in0=ni[:, 1:4, :],
                in1=r0.to_broadcast([P, 3, S]), op=OP.is_ge,
            )
            nc.vector.tensor_tensor(
                out=ge[:, 4:8, :], in0=ni[:, 4:8, :],
                in1=r1.to_broadcast([P, 4, S]), op=OP.is_ge,
            )
            nc.vector.tensor_tensor(
                out=cols[:, 1:8, :], in0=ni[:, 1:8, :], in1=ge[:, 1:8, :],
                op=OP.add,
            )

            ones = small.tile([P, 1], bf16, tag="ones")
            nc.vector.memset(ones[:], 1.0)
            biasc = small.tile([P, 1], fp32, tag="biasc")
            nc.vector.memset(biasc[:], float(-0.5 * LN_TEMP))

            # Pin the ACT function table (Square/Exp/Ln/Copy in one set).
            atl = mybir.InstLoadActFuncSet(
                name=nc.get_next_instruction_name(), ins=[], outs=[],
                act_func_set_id=6,
            )
            atl.engine = mybir.EngineType.Activation
            nc.scalar.add_instruction(atl)

            # ---------------- psum accumulators (split per half) --------
            ps_dots = [
                pp.tile([P, 16], fp32, tag="ps_dots0", name="ps_dots0"),
                pp.tile([P, 16], fp32, tag="ps_dots1", name="ps_dots1"),
            ]  # col = k*4 + jl
            ps_norm = [
                pp.tile([P, 16], fp32, tag="ps_norm0", name="ps_norm0"),
                pp.tile([P, 16], fp32, tag="ps_norm1", name="ps_norm1"),
            ]
            ps_own = pp.tile([P, JT], fp32, tag="ps_own")     # col = j
            ps_pos = pp.tile([P, 4], fp32, tag="ps_pos")      # col = jl

            def pe_reduce(src, ps, col, nb=1):
                """psum[:, col+b] = sum_d src[128d, c, b*128..] via matmuls."""
                for b in range(nb):
                    for c in range(C):
                        nc.tensor.matmul(
                            out=ps[:, col + b : col + b + 1],
                            lhsT=src[:, c, b * P : (b + 1) * P],
                            rhs=ones[:],
                            start=(c == 0),
                            stop=(c == C - 1),
                        )

            # ---------------- tiles ----------------
            sqa = big.tile([P, C, OWN], bf16, tag="sqa")
            prodp = big.tile([P, C, HLF], bf16, tag="prodp")
            def pieces(h, k):
                if h == 1 and k == 3:
                    return [(0, 256), (256, 128), (384, 128)]
                if h == 1 and k == 2:
                    return [(0, 256), (256, 256)]
                return [(0, HLF)]

            gt = {}
            prod = {}
            sq = {}
            for h in range(2):
                for k in range(R):
                    for lo, n in pieces(h, k):
                        key = (h, k, lo)
                        gt[key] = big.tile(
                            [P, C, n], bf16, tag=f"gt{h}{k}_{lo}",
                            name=f"gt{h}{k}_{lo}",
                        )
                        prod[key] = big.tile(
                            [P, C, n], bf16, tag=f"pr{h}{k}_{lo}",
                            name=f"pr{h}{k}_{lo}",
                        )
                        sq[key] = big.tile(
                            [P, C, n], bf16, tag=f"sq{h}{k}_{lo}",
                            name=f"sq{h}{k}_{lo}",
                        )

            # ---------------- gathers (Pool) ----------------
            for h in range(2):
                for k in range(R):
                    for lo, n in pieces(h, k):
                        nc.gpsimd.dma_gather(
                            out_ap=gt[(h, k, lo)][:],
                            in_ap=zbf[:],
                            idxs_ap=cols[
                                :, h * 4 + k, lo // 16 : (lo + n) // 16
                            ],
                            num_idxs=n,
                            num_idxs_reg=n,
                            elem_size=D,
                            transpose=True,
                        )

            # ---------------- own-row pipeline (early; ACT) -------------
            nc.scalar.activation(
                out=sqa[:, :, 0:HLF], in_=atp[:, :, 0:HLF], func=AF.Square
            )
            nc.scalar.activation(
                out=sqa[:, :, HLF:OWN], in_=atp[:, :, HLF:OWN], func=AF.Square
            )
            for j in range(JT):
                pe_reduce(sqa[:, :, j * P : (j + 1) * P], ps_own, j)
            # inva[p, j] = exp(-0.5*ln(ns_own) - 0.5*ln(temp))
            lna = small.tile([P, JT], fp32, tag="lna")
            inva = small.tile([P, JT], fp32, tag="inva")
            nc.scalar.activation(out=lna[:], in_=ps_own[:], func=AF.Ln)
            nc.scalar.activation(
                out=inva[:], in_=lna[:], func=AF.Exp, scale=-0.5, bias=biasc[:]
            )

            # ---------------- per-chunk compute + per-half softmax ------
            # square engine per piece: DVE gets the earliest chunks (it
            # starves waiting for the first gathers), ACT the mid chunks,
            # Pool the pieces that land after its gathers are done.
            sq_eng = {
                (0, 0, 0): "dve", (0, 1, 0): "dve",
                (0, 2, 0): "act", (0, 3, 0): "act",
                (1, 0, 0): "act", (1, 1, 0): "act",
                (1, 2, 0): "act", (1, 2, 256): "pool",
                (1, 3, 0): "pool", (1, 3, 256): "dve", (1, 3, 384): "act",
            }
            nll = small.tile([P, 16], fp32, tag="nll")  # cols: lp | lns
            lp = nll[:, 0:8]
            lns_all = nll[:, 8:16]
            explp = small.tile([P, JT], fp32, tag="explp")

            for h in range(2):
                a_h = atp[:, :, h * HLF : (h + 1) * HLF]
                for k in range(R):
                    for lo, n in pieces(h, k):
                        g = gt[(h, k, lo)]
                        pr = prod[(h, k, lo)]
                        s = sq[(h, k, lo)]
                        nc.vector.tensor_tensor(
                            out=pr[:],
                            in0=a_h[:, :, lo : lo + n],
                            in1=g[:],
                            op=OP.mult,
                        )
                        eng = sq_eng[(h, k, lo)]
                        if eng == "dve":
                            nc.vector.tensor_tensor(
                                out=s[:], in0=g[:], in1=g[:], op=OP.mult
                            )
                        elif eng == "pool":
                            nc.gpsimd.tensor_tensor(
                                out=s[:], in0=g[:], in1=g[:], op=OP.mult
                            )
                        else:
                            nc.scalar.activation(
                                out=s[:], in_=g[:], func=AF.Square
                            )
                        pe_reduce(
                            pr[:], ps_dots[h], k * 4 + lo // P, nb=n // P
                        )
                        pe_reduce(
                            s[:], ps_norm[h], k * 4 + lo // P, nb=n // P
                        )

                    if h == 0 and k == 1:
                        # own positives (atp fully loaded by now):
                        # prodp, pos dots, lp, exp(lp) -- all early
                        nc.vector.tensor_tensor(
                            out=prodp[:], in0=atp[:, :, 0:HLF],
                            in1=atp[:, :, HLF:OWN], op=OP.mult,
                        )
                        pe_reduce(prodp[:], ps_pos, 0, nb=4)
                        nc.vector.tensor_tensor(
                            out=lp[:, 0:4], in0=ps_pos[:], in1=inva[:, 0:4],
                            op=OP.mult,
                        )
                        nc.vector.tensor_tensor(
                            out=lp[:, 4:8], in0=ps_pos[:], in1=inva[:, 4:8],
                            op=OP.mult,
                        )
                        nc.vector.tensor_tensor(
                            out=lp[:, 0:4], in0=lp[:, 0:4], in1=inva[:, 4:8],
                            op=OP.mult,
                        )
                        nc.vector.tensor_tensor(
                            out=lp[:, 4:8], in0=lp[:, 4:8], in1=inva[:, 0:4],
                            op=OP.mult,
                        )
                        nc.scalar.activation(
                            out=explp[:], in_=lp[:], func=AF.Exp
                        )

                # ---- half-h softmax (psum read directly) ----
                nsg = ps_norm[h][:].rearrange("p (k j) -> p k j", k=R)
                dotg = ps_dots[h][:].rearrange("p (k j) -> p k j", k=R)
                lng = small.tile([P, R, 4], fp32, tag=f"lng{h}", name=f"lng{h}")
                invg = small.tile(
                    [P, R, 4], fp32, tag=f"invg{h}", name=f"invg{h}"
                )
                nc.scalar.activation(out=lng[:], in_=nsg, func=AF.Ln)
                nc.scalar.activation(
                    out=invg[:], in_=lng[:], func=AF.Exp, scale=-0.5,
                    bias=biasc[:],
                )
                # lg[p, jl, k] = dotg[p,k,jl] * invg[p,k,jl] * inva[p,h*4+jl]
                lg = small.tile([P, 4, R], fp32, tag=f"lg{h}", name=f"lg{h}")
                nc.vector.tensor_tensor(
                    out=lg[:],
                    in0=dotg.rearrange("p k j -> p j k"),
                    in1=invg[:].rearrange("p k j -> p j k"),
                    op=OP.mult,
                )
                nc.vector.tensor_tensor(
                    out=lg[:],
                    in0=lg[:],
                    in1=inva[:, h * 4 : (h + 1) * 4]
                    .rearrange("p (j o) -> p j o", o=1)
                    .to_broadcast([P, 4, R]),
                    op=OP.mult,
                )
                # sume = exp(lp) + sum_k exp(lg); logits <= 1/temp: no shift
                exlg = small.tile(
                    [P, 4, R], fp32, tag=f"exlg{h}", name=f"exlg{h}"
                )
                nc.scalar.activation(out=exlg[:], in_=lg[:], func=AF.Exp)
                sume = small.tile([P, 4], fp32, tag=f"sume{h}", name=f"sume{h}")
                nc.vector.tensor_reduce(
                    out=sume[:].rearrange("p (j o) -> p j o", o=1),
                    in_=exlg[:], axis=mybir.AxisListType.X, op=OP.add,
                )
                nc.vector.tensor_tensor(
                    out=sume[:], in0=sume[:],
                    in1=explp[:, h * 4 : (h + 1) * 4], op=OP.add,
                )
                nc.scalar.activation(
                    out=lns_all[:, h * 4 : (h + 1) * 4], in_=sume[:], func=AF.Ln
                )

            # ship lp|lns; host computes sum(lns - lp) (the unshard step).
            # Issued from ACT so it chains straight off the final Ln.
            nc.scalar.dma_start(out=out_nll[:], in_=nll[:])

    nc.finalize()
    return nc


def _grows(m):
    r = np.arange(HLF, dtype=np.int64)
    gi = m * HLF + r
    gj = B + m * HLF + r
    return np.concatenate([gi, gj])  # local row r -> global row


def make_in_maps(z_i, z_j, neg_idx):
    import ml_dtypes

    z = np.concatenate([np.asarray(z_i), np.asarray(z_j)], axis=0)
    zbf = np.ascontiguousarray(z.astype(ml_dtypes.bfloat16))
    neg_idx = np.asarray(neg_idx)
    S = HLF // 16
    in_maps = []
    for m in range(NCORES):
        g = _grows(m)  # [1024]
        atp = np.ascontiguousarray(
            zbf[g].reshape(OWN, C, P).transpose(2, 1, 0)
        )  # [P, C, OWN]
        idx = np.zeros((P, 10, S), dtype=np.int16)
        # gathered column i (0..511) of chunk (h,k) = own row h*512 + i;
        # its index lives at [i%16, h*4+k, i//16]
        i = np.arange(HLF)
        pl, sl = i % 16, i // 16
        for h in range(2):
            rows_g = g[h * HLF + i]  # global ids of this half's rows
            for k in range(R):
                idx[pl, h * 4 + k, sl] = neg_idx[rows_g, k].astype(np.int16)
            idx[pl, 8 + h, sl] = rows_g.astype(np.int16)
        in_maps.append({"zbf": zbf, "atp": atp, "idx": idx})
    return in_maps


def kernel(z_i, z_j, neg_idx, _bench=None):
    from concourse.bass_utils import run_bass_kernel_spmd

    if "nc" not in _CACHE:
        _CACHE["nc"] = build_nc()
    nc = _CACHE["nc"]
    in_maps = make_in_maps(z_i, z_j, neg_idx)
    core_ids = list(range(NCORES))
    kw = dict(_bench or {})
    r = run_bass_kernel_spmd(nc, in_maps, core_ids, **kw)
    if _bench is not None:
        _CACHE["last_results"] = r
    total = 0.0
    for m in range(NCORES):
        o = r.results[m]["nll"]
        total += np.sum(
            o[:, 8:16].astype(np.float64) - o[:, 0:8].astype(np.float64)
        )
    return np.float32(total / N)
